# revision 25
# baseline (speedup 1.0000x reference)
"""Distributed causal MultiHeadAttention kernel for 8 Trainium2 NeuronCores.

Problem: B=4, S=2048, D=1024, H=16 heads, dk=dv=64, causal mask, fp32 I/O.

Sharding: data-parallel over batch (4) x tensor-parallel over heads (2 groups
of 8) = 8 cores. Core c handles batch c//2 with heads (c%2)*8 .. (c%2)*8+7.
Each core computes a partial output [S, D] (its head group's contribution
through the corresponding w_o rows); the host sums the pair of partials per
batch (the "all-reduce" of the output projection, done host-side).

Device dataflow (all matmuls bf16 with fp32 PSUM accumulation):
  - Inputs arrive pre-cast to bf16 and pre-packed into the SBUF-native
    [128, chunk, free] partition-major layout host-side (layout prep only;
    all FLOPs happen on device, loads are single-descriptor-per-partition).
  - qT = wq.T @ xT -> [512, S] (head-major rows), same for kT.
  - v = xT.T @ wv -> [S, 512], stored with a constant 1.0 column appended
    per head ([S, 8, 65]) so the A@V matmul also produces the softmax row
    sums ("ones trick").
  - Scores computed transposed per head pair: S^T[k, q] = kT.T @ qT; the
    even/odd head of each 128-row chunk sits at partitions 0-63 / 64-127,
    so the two matmuls (contract dim 64) row-tile onto disjoint PE
    quadrants and run concurrently, writing the two banks of one
    [128, 2, 512] PSUM tile.
  - One exp per (pair, q-tile, k-chunk) on ScalarE straight out of PSUM
    (scale=1/8 folded in; no max subtraction - scores are O(1) bounded).
    Causal mask applied post-exp: fully-masked column ranges memset to 0,
    the triangular 128x128 block via gpsimd affine_select.
  - out^T[dv(+1), q] accumulated over k-chunks: lhsT = [V_h | 1], rhs = A^T.
    Row 64 of the PSUM result is the softmax denominator r[q]; its
    reciprocal is computed 128-lane-parallel via a DRAM-bounce reshape to
    [128, 4], then broadcast back over 64 partitions.
  - q-tile-outer loop: once a q-tile's columns of out^T are complete for
    all pairs, the output projection (out = oT.T @ wo) for those rows is
    issued immediately - PE fills the gaps of the ACT-bound attention.
"""

import numpy as np
import ml_dtypes

import concourse.bass as bass
import concourse.bacc as bacc
import concourse.mybir as mybir
import concourse.tile as tile
from concourse.bass_utils import run_bass_kernel_spmd

B, S, D = 4, 2048, 1024
H, DK = 16, 64
HL = 8              # heads handled per core
NHL = HL * DK       # 512 rows of head-dim per core
P = 128
NCORES = 8
ST = 512            # q-tile width (matmul free dim / PSUM bank)
NQT = S // ST       # 4
NKC = S // P        # 16 k chunks
MC = NHL // P       # 4 head-pair chunks
DC = D // P         # 8 chunks of D

FP32 = mybir.dt.float32
BF16 = mybir.dt.bfloat16
EXP = mybir.ActivationFunctionType.Exp


def _emit(tc):
    nc = tc.nc

    xqT = nc.dram_tensor("xqT", [P, DC, S], BF16, kind="ExternalInput").ap()
    xkT = nc.dram_tensor("xkT", [P, DC, S], BF16, kind="ExternalInput").ap()
    xvT = nc.dram_tensor("xvT", [P, DC, S], BF16, kind="ExternalInput").ap()
    wq = nc.dram_tensor("wq", [P, DC, NHL], BF16, kind="ExternalInput").ap()
    wk = nc.dram_tensor("wk", [P, DC, NHL], BF16, kind="ExternalInput").ap()
    wv = nc.dram_tensor("wv", [P, DC, NHL], BF16, kind="ExternalInput").ap()
    wo = nc.dram_tensor("wo", [P, MC, D], BF16, kind="ExternalInput").ap()
    out = nc.dram_tensor("out", [S, D], FP32, kind="ExternalOutput").ap()

    with (
        tc.tile_pool(name="sing", bufs=1) as sing,
        tc.tile_pool(name="apool", bufs=6) as apool,
        tc.tile_pool(name="rpool", bufs=4) as rpool,
        tc.tile_pool(name="outp", bufs=3) as outp,
        tc.tile_pool(name="xtp", bufs=2) as xtp,
        tc.tile_pool(name="dpool", bufs=4, space="DRAM") as dpool,
        tc.tile_pool(name="psS", bufs=2, space="PSUM") as psS,
        tc.tile_pool(name="psO", bufs=2, space="PSUM") as psO,
        tc.tile_pool(name="psP", bufs=2, space="PSUM") as psP,
    ):
        # ---- persistent SBUF tiles -------------------------------------
        wq_sb = sing.tile([P, DC, NHL], BF16, tag="wq_sb")
        wk_sb = sing.tile([P, DC, NHL], BF16, tag="wk_sb")
        wv_sb = sing.tile([P, DC, NHL], BF16, tag="wv_sb")
        wo_sb = sing.tile([P, MC, D], BF16, tag="wo_sb")
        qT = sing.tile([P, MC, S], BF16, tag="qT")
        kT = sing.tile([P, MC, S], BF16, tag="kT")
        v65 = sing.tile([P, NKC, HL, DK + 1], BF16, tag="v65")
        oT = sing.tile([P, MC, S], BF16, tag="oT")

        # ---- loads (per-chunk: single-descriptor-per-partition, and the
        # first projection matmuls can start after one chunk) -------------
        nc.gpsimd.memset(v65[:, :, :, DK : DK + 1], 1.0)
        for dc in range(DC):
            nc.sync.dma_start(wk_sb[:, dc, :], wk[:, dc, :])
        for dc in range(DC):
            nc.sync.dma_start(wq_sb[:, dc, :], wq[:, dc, :])
        for dc in range(DC):
            nc.sync.dma_start(wv_sb[:, dc, :], wv[:, dc, :])
        for c in range(MC):
            nc.sync.dma_start(wo_sb[:, c, :], wo[:, c, :])

        def load_xT(dram_ap):
            t = xtp.tile([P, DC, S], BF16, tag="xT", name="xT")
            for dc in range(DC):
                nc.sync.dma_start(t[:, dc, :], dram_ap[:, dc, :])
            return t

        # ---- projections: kT, qT  ([512, S], head-major rows) ----------
        for w_sb, x_dram, dst in ((wk_sb, xkT, kT), (wq_sb, xqT, qT)):
            x_sb = load_xT(x_dram)
            for st in range(NQT):
                for mc in range(MC):
                    ps = psP.tile([P, ST], FP32, tag="psP")
                    for dc in range(DC):
                        nc.tensor.matmul(
                            ps,
                            lhsT=w_sb[:, dc, mc * P : (mc + 1) * P],
                            rhs=x_sb[:, dc, st * ST : (st + 1) * ST],
                            start=(dc == 0),
                            stop=(dc == DC - 1),
                        )
                    nc.vector.tensor_copy(dst[:, mc, st * ST : (st + 1) * ST], ps)

        # ---- projection: v  ([S, 8, 65] with ones column) ---------------
        xvT_sb = load_xT(xvT)
        for sc in range(NKC):
            ps = psP.tile([P, ST], FP32, tag="psP")
            for dc in range(DC):
                nc.tensor.matmul(
                    ps,
                    lhsT=xvT_sb[:, dc, sc * P : (sc + 1) * P],
                    rhs=wv_sb[:, dc, :],
                    start=(dc == 0),
                    stop=(dc == DC - 1),
                )
            nc.vector.tensor_copy(
                v65[:, sc, :, 0:DK], ps.rearrange("p (h d) -> p h d", h=HL)
            )

        # ---- attention + folded output projection ------------------------
        # Masked-column skip: for k-chunk kc in q-tile qt, columns below
        # lo = (kc - 4*qt)*128 are entirely masked; scores/exp/A@V all skip
        # them. k-chunks are processed in groups of two so the PE switches
        # between the 64-row (scores) and 128-row (A@V) array modes half as
        # often (each switch drains the systolic array).
        def clo(kc, qt):
            j = kc - qt * (ST // P)
            return j * P if j > 0 else 0

        def emit_av(a_t, kc, oT_ps, pc, qt, nkc):
            lo = clo(kc, qt)
            for hh in range(2):
                nc.tensor.matmul(
                    oT_ps[hh][:, lo:ST],
                    lhsT=v65[:, kc, 2 * pc + hh, :],
                    rhs=a_t[:, hh, lo:ST],
                    start=(kc == 0),
                    stop=(kc == nkc - 1),
                )

        for qt in range(NQT):
            rdram_q = dpool.tile([2 * MC, ST], FP32, tag="rdq", name="rdram_q")
            for pc in range(MC):
                nkc = (qt + 1) * (ST // P)
                oT_ps = [
                    psO.tile([DK + 1, ST], FP32, tag="psO", name=f"psO_{hh}")
                    for hh in range(2)
                ]
                prev = []
                for g in range(nkc // 2):
                    for kc in (2 * g, 2 * g + 1):
                        lo = clo(kc, qt)
                        sps = psS.tile([P, 2, ST], FP32, tag="psS", name="sps")
                        for hh in range(2):
                            pp = hh * 64
                            nc.tensor.matmul(
                                sps[:, hh, lo:ST],
                                lhsT=kT[pp : pp + 64, pc, kc * P : (kc + 1) * P],
                                rhs=qT[pp : pp + 64, pc, qt * ST + lo : (qt + 1) * ST],
                                start=True,
                                stop=True,
                            )
                        a_t = apool.tile([P, 2, ST], BF16, tag="a", name="a_t")
                        nc.scalar.activation(
                            a_t[:, :, lo:ST], sps[:, :, lo:ST], EXP,
                            bias=0.0, scale=0.125,
                        )
                        if kc >= qt * (ST // P):
                            if lo > 0:
                                nc.gpsimd.memset(a_t[:, :, 0:lo], 0.0)
                            # triangular block: keep where q_local >= k_local
                            nc.gpsimd.affine_select(
                                out=a_t[:, :, lo : lo + P],
                                in_=a_t[:, :, lo : lo + P],
                                pattern=[[0, 2], [1, P]],
                                channel_multiplier=-1,
                                base=0,
                                compare_op=mybir.AluOpType.is_ge,
                                fill=0.0,
                            )
                        prev.append((a_t, kc))
                    while len(prev) > 2:
                        a_p, kc_p = prev.pop(0)
                        emit_av(a_p, kc_p, oT_ps, pc, qt, nkc)
                for a_p, kc_p in prev:
                    emit_av(a_p, kc_p, oT_ps, pc, qt, nkc)

                # release the accumulators fast: copy raw (unnormalized) oT
                # out and stash the softmax sums to DRAM; normalization is
                # done in bulk once per q-tile (below).
                for hh in range(2):
                    ps = oT_ps[hh]
                    rsb = rpool.tile([1, ST], FP32, tag="rsb")
                    nc.vector.tensor_copy(rsb, ps[DK : DK + 1, :])
                    nc.sync.dma_start(rdram_q[2 * pc + hh : 2 * pc + hh + 1, :], rsb)
                    nc.vector.tensor_copy(
                        oT[hh * 64 : (hh + 1) * 64, pc, qt * ST : (qt + 1) * ST],
                        ps[0:DK, :],
                    )

            # ---- bulk reciprocal of the 8 softmax-sum rows of this q-tile
            r128 = rpool.tile([P, 2 * MC * ST // P], FP32, tag="r128")
            nc.sync.dma_start(r128, rdram_q.rearrange("a (p f) -> (a p) f", p=16))
            ri128 = rpool.tile([P, 2 * MC * ST // P], FP32, tag="ri128")
            nc.vector.reciprocal(ri128, r128)
            rdram_i = dpool.tile([2 * MC, ST], FP32, tag="rdi", name="rdram_i")
            nc.sync.dma_start(rdram_i.rearrange("a (p f) -> (a p) f", p=16), ri128)
            for pc in range(MC):
                # one [128, ST] tile: rows 0-63 = 1/r of even head broadcast,
                # rows 64-127 = odd head (single DMA, partition dims fused)
                rrep = rpool.tile([P, ST], FP32, tag="rrep")
                for hh in range(2):
                    nc.sync.dma_start(
                        rrep[hh * 64 : (hh + 1) * 64, :],
                        rdram_i[2 * pc + hh : 2 * pc + hh + 1, :].to_broadcast(
                            (64, ST)
                        ),
                    )
                sl = oT[:, pc, qt * ST : (qt + 1) * ST]
                nc.vector.tensor_mul(sl, sl, rrep)

            # ---- output projection for this q-tile's rows ----------------
            for sc in range(qt * (ST // P), (qt + 1) * (ST // P)):
                for nt in range(D // ST):
                    ps = psP.tile([P, ST], FP32, tag="psP")
                    for c in range(MC):
                        nc.tensor.matmul(
                            ps,
                            lhsT=oT[:, c, sc * P : (sc + 1) * P],
                            rhs=wo_sb[:, c, nt * ST : (nt + 1) * ST],
                            start=(c == 0),
                            stop=(c == MC - 1),
                        )
                    ob = outp.tile([P, ST], FP32, tag="ob")
                    nc.vector.tensor_copy(ob, ps)
                    nc.sync.dma_start(
                        out[sc * P : (sc + 1) * P, nt * ST : (nt + 1) * ST], ob
                    )


_CACHE = {}


def build_nc():
    if "nc" not in _CACHE:
        # Bacc (not plain Bass): its finalize runs the pass pipeline that
        # splits multi-semaphore waits into event-semaphore/ldweights slots,
        # which walrus requires (max 1 wait per instruction on TRN2).
        nc = bacc.Bacc()
        with tile.TileContext(nc) as tc:
            _emit(tc)
        nc.finalize()
        _CACHE["nc"] = nc
    return _CACHE["nc"]


def make_in_maps(query, key, value, w_q, w_k, w_v, w_o):
    bf = ml_dtypes.bfloat16

    def packT(x):  # [S, D] fp32 -> xT packed [128, DC, S] bf16
        xb = np.asarray(x, np.float32).astype(bf)
        return np.ascontiguousarray(xb.T.reshape(DC, P, S).transpose(1, 0, 2))

    def packW(w):  # [D, NHL] -> [128, DC, NHL]
        wb = np.asarray(w, np.float32).astype(bf)
        return np.ascontiguousarray(wb.reshape(DC, P, NHL).transpose(1, 0, 2))

    def packWo(w):  # [NHL, D] -> [128, MC, D]
        wb = np.asarray(w, np.float32).astype(bf)
        return np.ascontiguousarray(wb.reshape(MC, P, D).transpose(1, 0, 2))

    query = np.asarray(query, np.float32)
    key = np.asarray(key, np.float32)
    value = np.asarray(value, np.float32)
    in_maps = []
    for c in range(NCORES):
        b, hg = divmod(c, 2)
        cols = slice(hg * NHL, (hg + 1) * NHL)
        in_maps.append(
            {
                "xqT": packT(query[b]),
                "xkT": packT(key[b]),
                "xvT": packT(value[b]),
                "wq": packW(np.asarray(w_q)[:, cols]),
                "wk": packW(np.asarray(w_k)[:, cols]),
                "wv": packW(np.asarray(w_v)[:, cols]),
                "wo": packWo(np.asarray(w_o)[cols, :]),
            }
        )
    return in_maps


def kernel(query, key, value, mask, w_q, w_k, w_v, w_o, **run_kwargs):
    nc = build_nc()
    in_maps = make_in_maps(query, key, value, w_q, w_k, w_v, w_o)
    res = run_bass_kernel_spmd(nc, in_maps, list(range(NCORES)), **run_kwargs)
    out = np.empty((B, S, D), np.float32)
    for b in range(B):
        out[b] = res.results[2 * b]["out"] + res.results[2 * b + 1]["out"]
    return out


# revision 26
# speedup vs baseline: 1.1454x; 1.1454x over previous
"""Distributed causal MultiHeadAttention kernel for 8 Trainium2 NeuronCores.

Problem: B=4, S=2048, D=1024, H=16 heads, dk=dv=64, causal mask, fp32 I/O.

Sharding: data-parallel over batch (4) x tensor-parallel over heads (2 groups
of 8) = 8 cores. Core c handles batch c//2 with heads (c%2)*8 .. (c%2)*8+7.
Each core computes a partial output [S, D] (its head group's contribution
through the corresponding w_o rows); the host sums the pair of partials per
batch (the "all-reduce" of the output projection, done host-side).

Device dataflow (all matmuls bf16 with fp32 PSUM accumulation):
  - Inputs arrive pre-cast to bf16 and pre-packed into the SBUF-native
    [128, chunk, free] partition-major layout host-side (layout prep only;
    all FLOPs happen on device, loads are single-descriptor-per-partition).
  - qT = wq.T @ xT -> [512, S] (head-major rows), same for kT.
  - v = xT.T @ wv -> [S, 512], stored with a constant 1.0 column appended
    per head ([S, 8, 65]) so the A@V matmul also produces the softmax row
    sums ("ones trick").
  - Scores computed transposed per head pair: S^T[k, q] = kT.T @ qT; the
    even/odd head of each 128-row chunk sits at partitions 0-63 / 64-127,
    so the two matmuls (contract dim 64) row-tile onto disjoint PE
    quadrants and run concurrently, writing the two banks of one
    [128, 2, 512] PSUM tile.
  - One exp per (pair, q-tile, k-chunk) on ScalarE straight out of PSUM
    (scale=1/8 folded in; no max subtraction - scores are O(1) bounded).
    Causal mask applied post-exp: fully-masked column ranges memset to 0,
    the triangular 128x128 block via gpsimd affine_select.
  - out^T[dv(+1), q] accumulated over k-chunks: lhsT = [V_h | 1], rhs = A^T.
    Row 64 of the PSUM result is the softmax denominator r[q]; its
    reciprocal is computed 128-lane-parallel via a DRAM-bounce reshape to
    [128, 4], then broadcast back over 64 partitions.
  - q-tile-outer loop: once a q-tile's columns of out^T are complete for
    all pairs, the output projection (out = oT.T @ wo) for those rows is
    issued immediately - PE fills the gaps of the ACT-bound attention.
"""

import numpy as np
import ml_dtypes

import concourse.bass as bass
import concourse.bacc as bacc
import concourse.mybir as mybir
import concourse.tile as tile
from concourse.bass_utils import run_bass_kernel_spmd

B, S, D = 4, 2048, 1024
H, DK = 16, 64
HL = 8              # heads handled per core
NHL = HL * DK       # 512 rows of head-dim per core
P = 128
NCORES = 8
ST = 512            # q-tile width (matmul free dim / PSUM bank)
NQT = S // ST       # 4
NKC = S // P        # 16 k chunks
MC = NHL // P       # 4 head-pair chunks
DC = D // P         # 8 chunks of D

FP32 = mybir.dt.float32
BF16 = mybir.dt.bfloat16
EXP = mybir.ActivationFunctionType.Exp


def _emit(tc):
    nc = tc.nc

    xqT = nc.dram_tensor("xqT", [P, DC, S], BF16, kind="ExternalInput").ap()
    xkT = nc.dram_tensor("xkT", [P, DC, S], BF16, kind="ExternalInput").ap()
    xvT = nc.dram_tensor("xvT", [P, DC, S], BF16, kind="ExternalInput").ap()
    wq = nc.dram_tensor("wq", [P, DC, NHL], BF16, kind="ExternalInput").ap()
    wk = nc.dram_tensor("wk", [P, DC, NHL], BF16, kind="ExternalInput").ap()
    wv = nc.dram_tensor("wv", [P, DC, NHL], BF16, kind="ExternalInput").ap()
    wo = nc.dram_tensor("wo", [P, MC, D], BF16, kind="ExternalInput").ap()
    out = nc.dram_tensor("out", [S, D], FP32, kind="ExternalOutput").ap()

    with (
        tc.tile_pool(name="sing", bufs=1) as sing,
        tc.tile_pool(name="apool", bufs=6) as apool,
        tc.tile_pool(name="rpool", bufs=4) as rpool,
        tc.tile_pool(name="outp", bufs=3) as outp,
        tc.tile_pool(name="xtp", bufs=2) as xtp,
        tc.tile_pool(name="dpool", bufs=4, space="DRAM") as dpool,
        tc.tile_pool(name="psS", bufs=2, space="PSUM") as psS,
        tc.tile_pool(name="psO", bufs=2, space="PSUM") as psO,
        tc.tile_pool(name="psP", bufs=2, space="PSUM") as psP,
    ):
        # ---- persistent SBUF tiles -------------------------------------
        wq_sb = sing.tile([P, DC, NHL], BF16, tag="wq_sb")
        wk_sb = sing.tile([P, DC, NHL], BF16, tag="wk_sb")
        wv_sb = sing.tile([P, DC, NHL], BF16, tag="wv_sb")
        wo_sb = sing.tile([P, MC, D], BF16, tag="wo_sb")
        qT = sing.tile([P, MC, S], BF16, tag="qT")
        kT = sing.tile([P, MC, S], BF16, tag="kT")
        v65 = sing.tile([P, NKC, HL, DK + 1], BF16, tag="v65")
        oT = sing.tile([P, MC, S], BF16, tag="oT")

        # ---- loads (per-chunk: single-descriptor-per-partition, and the
        # first projection matmuls can start after one chunk) -------------
        nc.gpsimd.memset(v65[:, :, :, DK : DK + 1], 1.0)
        for dc in range(DC):
            nc.sync.dma_start(wk_sb[:, dc, :], wk[:, dc, :])
        for dc in range(DC):
            nc.sync.dma_start(wq_sb[:, dc, :], wq[:, dc, :])
        for dc in range(DC):
            nc.sync.dma_start(wv_sb[:, dc, :], wv[:, dc, :])
        for c in range(MC):
            nc.sync.dma_start(wo_sb[:, c, :], wo[:, c, :])

        def load_xT(dram_ap):
            t = xtp.tile([P, DC, S], BF16, tag="xT", name="xT")
            for dc in range(DC):
                nc.sync.dma_start(t[:, dc, :], dram_ap[:, dc, :])
            return t

        # ---- projections: kT, qT  ([512, S], head-major rows) ----------
        for w_sb, x_dram, dst in ((wk_sb, xkT, kT), (wq_sb, xqT, qT)):
            x_sb = load_xT(x_dram)
            for st in range(NQT):
                for mc in range(MC):
                    ps = psP.tile([P, ST], FP32, tag="psP")
                    for dc in range(DC):
                        nc.tensor.matmul(
                            ps,
                            lhsT=w_sb[:, dc, mc * P : (mc + 1) * P],
                            rhs=x_sb[:, dc, st * ST : (st + 1) * ST],
                            start=(dc == 0),
                            stop=(dc == DC - 1),
                        )
                    nc.vector.tensor_copy(dst[:, mc, st * ST : (st + 1) * ST], ps)

        # ---- projection: v  ([S, 8, 65] with ones column) ---------------
        xvT_sb = load_xT(xvT)
        for sc in range(NKC):
            ps = psP.tile([P, ST], FP32, tag="psP")
            for dc in range(DC):
                nc.tensor.matmul(
                    ps,
                    lhsT=xvT_sb[:, dc, sc * P : (sc + 1) * P],
                    rhs=wv_sb[:, dc, :],
                    start=(dc == 0),
                    stop=(dc == DC - 1),
                )
            nc.vector.tensor_copy(
                v65[:, sc, :, 0:DK], ps.rearrange("p (h d) -> p h d", h=HL)
            )

        # ---- attention + folded output projection ------------------------
        # Masked-column skip: for k-chunk kc in q-tile qt, columns below
        # lo = (kc - 4*qt)*128 are entirely masked; scores/exp/A@V all skip
        # them. k-chunks are processed in groups of two so the PE switches
        # between the 64-row (scores) and 128-row (A@V) array modes half as
        # often (each switch drains the systolic array).
        def clo(kc, qt):
            j = kc - qt * (ST // P)
            return j * P if j > 0 else 0

        def emit_av(a_t, kc, oT_ps, pc, qt, nkc):
            lo = clo(kc, qt)
            for hh in range(2):
                nc.tensor.matmul(
                    oT_ps[hh][:, lo:ST],
                    lhsT=v65[:, kc, 2 * pc + hh, :],
                    rhs=a_t[:, hh, lo:ST],
                    start=(kc == 0),
                    stop=(kc == nkc - 1),
                )

        for qt in range(NQT):
            rdram_q = dpool.tile([2 * MC, ST], FP32, tag="rdq", name="rdram_q")
            for pc in range(MC):
                nkc = (qt + 1) * (ST // P)
                oT_ps = [
                    psO.tile([DK + 1, ST], FP32, tag="psO", name=f"psO_{hh}")
                    for hh in range(2)
                ]
                prev = []
                for kc in range(nkc):
                    lo = clo(kc, qt)
                    sps = psS.tile([P, 2, ST], FP32, tag="psS", name="sps")
                    for hh in range(2):
                        pp = hh * 64
                        nc.tensor.matmul(
                            sps[:, hh, lo:ST],
                            lhsT=kT[pp : pp + 64, pc, kc * P : (kc + 1) * P],
                            rhs=qT[pp : pp + 64, pc, qt * ST + lo : (qt + 1) * ST],
                            start=True,
                            stop=True,
                        )
                    a_t = apool.tile([P, 2, ST], BF16, tag="a", name="a_t")
                    nc.scalar.activation(
                        a_t[:, :, lo:ST], sps[:, :, lo:ST], EXP,
                        bias=0.0, scale=0.125,
                    )
                    if kc >= qt * (ST // P):
                        if lo > 0:
                            nc.gpsimd.memset(a_t[:, :, 0:lo], 0.0)
                        # triangular block: keep where q_local >= k_local
                        nc.gpsimd.affine_select(
                            out=a_t[:, :, lo : lo + P],
                            in_=a_t[:, :, lo : lo + P],
                            pattern=[[0, 2], [1, P]],
                            channel_multiplier=-1,
                            base=0,
                            compare_op=mybir.AluOpType.is_ge,
                            fill=0.0,
                        )
                    prev.append((a_t, kc))
                    if len(prev) > 1:
                        a_p, kc_p = prev.pop(0)
                        emit_av(a_p, kc_p, oT_ps, pc, qt, nkc)
                for a_p, kc_p in prev:
                    emit_av(a_p, kc_p, oT_ps, pc, qt, nkc)

                # release the accumulators fast: copy raw (unnormalized) oT
                # out and stash the softmax sums to DRAM; normalization is
                # done in bulk once per q-tile (below).
                for hh in range(2):
                    ps = oT_ps[hh]
                    rsb = rpool.tile([1, ST], FP32, tag="rsb")
                    nc.vector.tensor_copy(rsb, ps[DK : DK + 1, :])
                    nc.sync.dma_start(rdram_q[2 * pc + hh : 2 * pc + hh + 1, :], rsb)
                    nc.vector.tensor_copy(
                        oT[hh * 64 : (hh + 1) * 64, pc, qt * ST : (qt + 1) * ST],
                        ps[0:DK, :],
                    )

            # ---- bulk reciprocal of the 8 softmax-sum rows of this q-tile
            r128 = rpool.tile([P, 2 * MC * ST // P], FP32, tag="r128")
            nc.sync.dma_start(r128, rdram_q.rearrange("a (p f) -> (a p) f", p=16))
            ri128 = rpool.tile([P, 2 * MC * ST // P], FP32, tag="ri128")
            nc.vector.reciprocal(ri128, r128)
            rdram_i = dpool.tile([2 * MC, ST], FP32, tag="rdi", name="rdram_i")
            nc.sync.dma_start(rdram_i.rearrange("a (p f) -> (a p) f", p=16), ri128)
            for pc in range(MC):
                # one [128, ST] tile: rows 0-63 = 1/r of even head broadcast,
                # rows 64-127 = odd head (single DMA, partition dims fused)
                rrep = rpool.tile([P, ST], FP32, tag="rrep")
                for hh in range(2):
                    nc.sync.dma_start(
                        rrep[hh * 64 : (hh + 1) * 64, :],
                        rdram_i[2 * pc + hh : 2 * pc + hh + 1, :].to_broadcast(
                            (64, ST)
                        ),
                    )
                sl = oT[:, pc, qt * ST : (qt + 1) * ST]
                nc.vector.tensor_mul(sl, sl, rrep)

            # ---- output projection for this q-tile's rows ----------------
            for sc in range(qt * (ST // P), (qt + 1) * (ST // P)):
                for nt in range(D // ST):
                    ps = psP.tile([P, ST], FP32, tag="psP")
                    for c in range(MC):
                        nc.tensor.matmul(
                            ps,
                            lhsT=oT[:, c, sc * P : (sc + 1) * P],
                            rhs=wo_sb[:, c, nt * ST : (nt + 1) * ST],
                            start=(c == 0),
                            stop=(c == MC - 1),
                        )
                    ob = outp.tile([P, ST], FP32, tag="ob")
                    nc.vector.tensor_copy(ob, ps)
                    nc.sync.dma_start(
                        out[sc * P : (sc + 1) * P, nt * ST : (nt + 1) * ST], ob
                    )


_CACHE = {}


def build_nc():
    if "nc" not in _CACHE:
        # Bacc (not plain Bass): its finalize runs the pass pipeline that
        # splits multi-semaphore waits into event-semaphore/ldweights slots,
        # which walrus requires (max 1 wait per instruction on TRN2).
        nc = bacc.Bacc()
        with tile.TileContext(nc) as tc:
            _emit(tc)
        nc.finalize()
        _CACHE["nc"] = nc
    return _CACHE["nc"]


def make_in_maps(query, key, value, w_q, w_k, w_v, w_o):
    bf = ml_dtypes.bfloat16

    def packT(x):  # [S, D] fp32 -> xT packed [128, DC, S] bf16
        xb = np.asarray(x, np.float32).astype(bf)
        return np.ascontiguousarray(xb.T.reshape(DC, P, S).transpose(1, 0, 2))

    def packW(w):  # [D, NHL] -> [128, DC, NHL]
        wb = np.asarray(w, np.float32).astype(bf)
        return np.ascontiguousarray(wb.reshape(DC, P, NHL).transpose(1, 0, 2))

    def packWo(w):  # [NHL, D] -> [128, MC, D]
        wb = np.asarray(w, np.float32).astype(bf)
        return np.ascontiguousarray(wb.reshape(MC, P, D).transpose(1, 0, 2))

    query = np.asarray(query, np.float32)
    key = np.asarray(key, np.float32)
    value = np.asarray(value, np.float32)
    in_maps = []
    for c in range(NCORES):
        b, hg = divmod(c, 2)
        cols = slice(hg * NHL, (hg + 1) * NHL)
        in_maps.append(
            {
                "xqT": packT(query[b]),
                "xkT": packT(key[b]),
                "xvT": packT(value[b]),
                "wq": packW(np.asarray(w_q)[:, cols]),
                "wk": packW(np.asarray(w_k)[:, cols]),
                "wv": packW(np.asarray(w_v)[:, cols]),
                "wo": packWo(np.asarray(w_o)[cols, :]),
            }
        )
    return in_maps


def kernel(query, key, value, mask, w_q, w_k, w_v, w_o, **run_kwargs):
    nc = build_nc()
    in_maps = make_in_maps(query, key, value, w_q, w_k, w_v, w_o)
    res = run_bass_kernel_spmd(nc, in_maps, list(range(NCORES)), **run_kwargs)
    out = np.empty((B, S, D), np.float32)
    for b in range(B):
        out[b] = res.results[2 * b]["out"] + res.results[2 * b + 1]["out"]
    return out
